# revision 7
# baseline (speedup 1.0000x reference)
"""Self-contained Trainium2 kernel for nn_GATNetSelectiveResidualsUpdated.

GATv2 layer + MLP head + pairwise-distance output, distributed over 8
NeuronCores: dst-nodes (and the cdist row block) are sharded per core,
edges grouped by dst block, xl table built redundantly per core, y
all-gathered on-device for the cdist columns.

kernel(**inputs) takes the FULL inputs (as produced by setup_inputs())
and returns the FULL [12288, 12288] float32 output.
"""
import numpy as np
import ml_dtypes
from contextlib import ExitStack

import concourse.bass as bass
import concourse.bacc as bacc
import concourse.mybir as mybir
from concourse import tile
from concourse.masks import make_identity

dt = mybir.dt
AF = mybir.ActivationFunctionType
ALU = mybir.AluOpType

N = 12288
F = 512          # H*C
C = 256
NC = 8
ND = N // NC     # 1536 nodes per core
NB = ND // 128   # 12 dst blocks per core
TB = 27          # edge tiles per block (padded)
NM = N // 128    # 96 m-tiles for full table
EPS = 1e-5

bf = dt.bfloat16
f32 = dt.float32

_cache = {}


def build_program():
    nc = bacc.Bacc(None, target_bir_lowering=False)
    xT = nc.dram_tensor("xT", [F, N], bf, kind="ExternalInput")
    xTl = nc.dram_tensor("xTl", [F, ND], bf, kind="ExternalInput")
    wl = nc.dram_tensor("wl", [F, F], bf, kind="ExternalInput")
    blr = nc.dram_tensor("blr", [1, F], bf, kind="ExternalInput")
    wr = nc.dram_tensor("wr", [F, F], bf, kind="ExternalInput")
    brr = nc.dram_tensor("brr", [1, F], bf, kind="ExternalInput")
    attT = nc.dram_tensor("attT", [F, 2], bf, kind="ExternalInput")
    biasc = nc.dram_tensor("biasc", [1, F], bf, kind="ExternalInput")
    sidx = nc.dram_tensor("sidx", [NB, 128, TB], dt.int32, kind="ExternalInput")
    mt_in = nc.dram_tensor("mt_in", [NB, 128, TB, 128], bf, kind="ExternalInput")
    m01_in = nc.dram_tensor("m01_in", [NB, 128, TB, 128], bf, kind="ExternalInput")
    # MLP weights
    wda = nc.dram_tensor("wda", [F, 256], bf, kind="ExternalInput")
    bda = nc.dram_tensor("bda", [1, 256], bf, kind="ExternalInput")
    wala = nc.dram_tensor("wala", [F, 256], bf, kind="ExternalInput")
    bala = nc.dram_tensor("bala", [1, 256], bf, kind="ExternalInput")
    wd1 = nc.dram_tensor("wd1", [256, 128], bf, kind="ExternalInput")
    bd1 = nc.dram_tensor("bd1", [1, 128], bf, kind="ExternalInput")
    wal1 = nc.dram_tensor("wal1", [256, 128], bf, kind="ExternalInput")
    bal1 = nc.dram_tensor("bal1", [1, 128], bf, kind="ExternalInput")
    wd2 = nc.dram_tensor("wd2", [128, 64], bf, kind="ExternalInput")
    bd2 = nc.dram_tensor("bd2", [1, 64], bf, kind="ExternalInput")
    wd3 = nc.dram_tensor("wd3", [64, 3], bf, kind="ExternalInput")
    bd3 = nc.dram_tensor("bd3", [1, 3], bf, kind="ExternalInput")
    # LN affine params (f32)
    lnga = nc.dram_tensor("lnga", [1, 256], f32, kind="ExternalInput")
    lnba = nc.dram_tensor("lnba", [1, 256], f32, kind="ExternalInput")
    lng1 = nc.dram_tensor("lng1", [1, 128], f32, kind="ExternalInput")
    lnb1 = nc.dram_tensor("lnb1", [1, 128], f32, kind="ExternalInput")
    lng2 = nc.dram_tensor("lng2", [1, 64], f32, kind="ExternalInput")
    lnb2 = nc.dram_tensor("lnb2", [1, 64], f32, kind="ExternalInput")

    table = nc.dram_tensor("table", [N, 514], bf, kind="Internal")
    cc_in = nc.dram_tensor("cc_in", [5, ND], f32, kind="Internal")
    cc_out = nc.dram_tensor("cc_out", [NC * 5, ND], f32, kind="Internal",
                            addr_space="Shared")
    out = nc.dram_tensor("out", [ND, N], f32, kind="ExternalOutput")

    with tile.TileContext(nc) as tc, ExitStack() as ctx:
        cpool = ctx.enter_context(tc.tile_pool(name="const", bufs=1))
        ident = cpool.tile([128, 128], bf)
        make_identity(nc, ident[:])
        identf = cpool.tile([128, 128], f32)
        make_identity(nc, identf[:])
        ones1 = cpool.tile([1, 128], bf)
        nc.vector.memset(ones1[:], 1.0)
        eps_sb = cpool.tile([128, 1], f32)
        nc.vector.memset(eps_sb[:], EPS)
        att_sb = cpool.tile([128, 4, 2], bf)
        nc.sync.dma_start(out=att_sb[:], in_=attT[:].rearrange("(a p) h -> p a h", p=128))
        biasc_sb = cpool.tile([1, F], bf)
        nc.sync.dma_start(out=biasc_sb[:], in_=biasc[:])

        # broadcast constants to all partitions via rank-1 matmuls
        def bcast_f32(row_dram, width, name):
            row_sb = cpool.tile([1, width], f32, name=f"{name}_row")
            nc.sync.dma_start(out=row_sb[:], in_=row_dram[:])
            row_bf = cpool.tile([1, width], bf, name=f"{name}_bf")
            nc.vector.tensor_copy(out=row_bf[:], in_=row_sb[:])
            ps = initps.tile([128, width], f32, tag="initps")
            nc.tensor.matmul(out=ps[:], lhsT=ones1[:], rhs=row_bf[:], start=True, stop=True)
            bcast = cpool.tile([128, width], f32, name=f"{name}_bc")
            nc.vector.tensor_copy(out=bcast[:], in_=ps[:])
            # exact affine correction: bcast = bcast - bf16err  (bf16 round-trip
            # loses precision; add the residual from the f32 row)
            return bcast, row_sb

        with tc.tile_pool(name="initps", bufs=2, space="PSUM") as initps:
            ps = initps.tile([128, F], f32, tag="initps")
            nc.tensor.matmul(out=ps[:], lhsT=ones1[:], rhs=biasc_sb[:], start=True, stop=True)
            biasc_bc = cpool.tile([128, F], f32)
            nc.vector.tensor_copy(out=biasc_bc[:], in_=ps[:])
            ga_bc, _ = bcast_f32(lnga, 256, "ga")
            ba_bc, _ = bcast_f32(lnba, 256, "ba")
            g1_bc, _ = bcast_f32(lng1, 128, "g1")
            b1_bc, _ = bcast_f32(lnb1, 128, "b1")
            g2_bc, _ = bcast_f32(lng2, 64, "g2")
            b2_bc, _ = bcast_f32(lnb2, 64, "b2")

        # ---------- Phase A1: full xl table ----------
        wl_sb = cpool.tile([128, 4, F], bf)
        for k in range(4):
            nc.sync.dma_start(out=wl_sb[:, k, :], in_=wl[k * 128:(k + 1) * 128, :])
        bl_sb = cpool.tile([1, F], bf)
        nc.sync.dma_start(out=bl_sb[:], in_=blr[:])
        wr_sb = cpool.tile([128, 4, F], bf)
        for k in range(4):
            nc.sync.dma_start(out=wr_sb[:, k, :], in_=wr[k * 128:(k + 1) * 128, :])
        br_sb = cpool.tile([1, F], bf)
        nc.sync.dma_start(out=br_sb[:], in_=brr[:])

        xT_v = xT[:].rearrange("(a p) n -> p a n", p=128)
        with tc.tile_pool(name="a1x", bufs=3) as xa_pool, \
             tc.tile_pool(name="a1ps", bufs=2, space="PSUM") as ps_a, \
             tc.tile_pool(name="a1bld", bufs=3) as bld_pool:
            for m in range(NM):
                xa = xa_pool.tile([128, 4, 128], bf)
                nc.sync.dma_start(out=xa[:], in_=xT_v[:, :, m * 128:(m + 1) * 128])
                ps = ps_a.tile([128, F], f32)
                for k in range(4):
                    nc.tensor.matmul(out=ps[:], lhsT=xa[:, k, :], rhs=wl_sb[:, k, :],
                                     start=(k == 0), stop=False)
                nc.tensor.matmul(out=ps[:], lhsT=ones1[:], rhs=bl_sb[:], start=False, stop=True)
                bld = bld_pool.tile([128, 514], bf)
                nc.vector.memset(bld[:, 256:257], 1.0)
                nc.vector.memset(bld[:, 513:514], 1.0)
                nc.scalar.activation(out=bld[:, 0:256], in_=ps[:, 0:256], func=AF.Copy)
                nc.scalar.activation(out=bld[:, 257:513], in_=ps[:, 256:512], func=AF.Copy)
                nc.sync.dma_start(out=table[m * 128:(m + 1) * 128, :], in_=bld[:])

        # ---------- Phase A2: local xr (node-major, SBUF-resident) ----------
        vb_pool = ctx.enter_context(tc.tile_pool(name="vb", bufs=NB))
        vbs = []
        xTl_v = xTl[:].rearrange("(a p) n -> p a n", p=128)
        with tc.tile_pool(name="a2x", bufs=2) as xa2_pool, \
             tc.tile_pool(name="a2ps", bufs=2, space="PSUM") as ps_a2:
            for b in range(NB):
                xa = xa2_pool.tile([128, 4, 128], bf)
                nc.sync.dma_start(out=xa[:], in_=xTl_v[:, :, b * 128:(b + 1) * 128])
                ps = ps_a2.tile([128, F], f32)
                for k in range(4):
                    nc.tensor.matmul(out=ps[:], lhsT=xa[:, k, :], rhs=wr_sb[:, k, :],
                                     start=(k == 0), stop=False)
                nc.tensor.matmul(out=ps[:], lhsT=ones1[:], rhs=br_sb[:], start=False, stop=True)
                vb = vb_pool.tile([128, F], bf, tag="vb", bufs=NB)
                nc.scalar.activation(out=vb[:], in_=ps[:], func=AF.Copy)
                vbs.append(vb)

        # h1T storage (transposed GAT output, bf16, persistent)
        h1T_pool = ctx.enter_context(tc.tile_pool(name="h1T", bufs=NB))
        h1Ts = []

        # ---------- Phase B: edge loop ----------
        with tc.tile_pool(name="mtp", bufs=2) as mt_pool, \
             tc.tile_pool(name="m01p", bufs=2) as m01_pool, \
             tc.tile_pool(name="sip", bufs=2) as si_pool, \
             tc.tile_pool(name="xgp", bufs=3) as xg_pool, \
             tc.tile_pool(name="pmp", bufs=2, space="PSUM") as pm_pool, \
             tc.tile_pool(name="tpsp", bufs=2, space="PSUM") as tps_pool, \
             tc.tile_pool(name="esp", bufs=2, space="PSUM") as es_ps, \
             tc.tile_pool(name="Up", bufs=2, space="PSUM") as U_pool, \
             tc.tile_pool(name="rsbp", bufs=2) as rsb_pool, \
             tc.tile_pool(name="rTp", bufs=2) as rT_pool, \
             tc.tile_pool(name="essb", bufs=2) as essb_pool, \
             tc.tile_pool(name="szp", bufs=2) as sz_pool, \
             tc.tile_pool(name="h1p", bufs=2) as h1_pool:
            for b in range(NB):
                mt_sb = mt_pool.tile([128, TB, 128], bf)
                nc.sync.dma_start(out=mt_sb[:], in_=mt_in[b])
                m01_sb = m01_pool.tile([128, TB, 128], bf)
                nc.sync.dma_start(out=m01_sb[:], in_=m01_in[b])
                si_sb = si_pool.tile([128, TB], dt.int32)
                nc.sync.dma_start(out=si_sb[:], in_=sidx[b])
                U0 = U_pool.tile([128, 257], f32, tag="U")
                U1 = U_pool.tile([128, 257], f32, tag="U")
                for t in range(TB):
                    xg = xg_pool.tile([128, 514], bf)
                    nc.gpsimd.indirect_dma_start(
                        out=xg[:], out_offset=None, in_=table[:],
                        in_offset=bass.IndirectOffsetOnAxis(ap=si_sb[:, t:t + 1], axis=0))
                    pm = pm_pool.tile([128, F], f32)
                    xg_m = xg[:].rearrange("p (h x) -> p h x", x=257)[:, :, 0:256]
                    nc.tensor.matmul(out=pm[:], lhsT=ident[:], rhs=xg_m, start=True, stop=False)
                    nc.tensor.matmul(out=pm[:], lhsT=m01_sb[:, t, :], rhs=vbs[b][:],
                                     start=False, stop=True)
                    r_sb = rsb_pool.tile([128, F], bf)
                    nc.scalar.activation(out=r_sb[:], in_=pm[:], func=AF.Prelu, alpha=0.2)
                    rT_ps = tps_pool.tile([128, F], bf)
                    for k in range(4):
                        nc.tensor.transpose(out=rT_ps[:, k * 128:(k + 1) * 128],
                                            in_=r_sb[:, k * 128:(k + 1) * 128],
                                            identity=ident[:])
                    rT_sb = rT_pool.tile([128, F], bf)
                    nc.vector.tensor_copy(out=rT_sb[:], in_=rT_ps[:])
                    eT = es_ps.tile([2, 128], f32, tag="es")
                    for k in range(4):
                        nc.tensor.matmul(out=eT[:], lhsT=att_sb[:, k, :],
                                         rhs=rT_sb[:, k * 128:(k + 1) * 128],
                                         start=(k == 0), stop=(k == 3))
                    zT = essb_pool.tile([2, 128], bf, tag="zT")
                    nc.scalar.activation(out=zT[:], in_=eT[:], func=AF.Exp)
                    zps = es_ps.tile([128, 2], bf, tag="es")
                    nc.tensor.transpose(out=zps[:], in_=zT[:], identity=ident[:2, :2])
                    z_sb = essb_pool.tile([128, 2], f32, tag="zs")
                    nc.vector.tensor_copy(out=z_sb[:], in_=zps[:])
                    sz = sz_pool.tile([128, 2, 128], bf)
                    nc.vector.tensor_scalar(out=sz[:, 0, :], in0=mt_sb[:, t, :],
                                            scalar1=z_sb[:, 0:1], scalar2=None, op0=ALU.mult)
                    nc.vector.tensor_scalar(out=sz[:, 1, :], in0=mt_sb[:, t, :],
                                            scalar1=z_sb[:, 1:2], scalar2=None, op0=ALU.mult)
                    nc.tensor.matmul(out=U0[:], lhsT=sz[:, 0, :], rhs=xg[:, 0:257],
                                     start=(t == 0), stop=(t == TB - 1))
                    nc.tensor.matmul(out=U1[:], lhsT=sz[:, 1, :], rhs=xg[:, 257:514],
                                     start=(t == 0), stop=(t == TB - 1))
                # h1 assembly for block b
                rc = essb_pool.tile([128, 2], f32, tag="rc")
                nc.vector.reciprocal(out=rc[:, 0:1], in_=U0[:, 256:257])
                nc.vector.reciprocal(out=rc[:, 1:2], in_=U1[:, 256:257])
                h1a = h1_pool.tile([128, F], f32, tag="h1a")
                nc.vector.tensor_scalar(out=h1a[:, 0:256], in0=U0[:, 0:256],
                                        scalar1=rc[:, 0:1], scalar2=None, op0=ALU.mult)
                nc.vector.tensor_scalar(out=h1a[:, 256:512], in0=U1[:, 0:256],
                                        scalar1=rc[:, 1:2], scalar2=None, op0=ALU.mult)
                h1b = h1_pool.tile([128, F], f32, tag="h1b")
                nc.vector.tensor_tensor(out=h1b[:], in0=h1a[:], in1=biasc_bc[:], op=ALU.add)
                h1f = h1_pool.tile([128, F], bf, tag="h1f")
                nc.scalar.activation(out=h1f[:], in_=h1b[:], func=AF.Relu)
                h1T_ps = tps_pool.tile([128, F], bf, tag="rT_ps")
                for k in range(4):
                    nc.tensor.transpose(out=h1T_ps[:, k * 128:(k + 1) * 128],
                                        in_=h1f[:, k * 128:(k + 1) * 128],
                                        identity=ident[:])
                h1T = h1T_pool.tile([128, 4, 128], bf, tag="h1T", bufs=NB)
                nc.vector.tensor_copy(out=h1T[:], in_=h1T_ps[:])
                h1Ts.append(h1T)

        # ---------- Phase C: MLP head ----------
        # load MLP weights
        wda_sb = cpool.tile([128, 4, 256], bf)
        for k in range(4):
            nc.sync.dma_start(out=wda_sb[:, k, :], in_=wda[k * 128:(k + 1) * 128, :])
        wala_sb = cpool.tile([128, 4, 256], bf)
        for k in range(4):
            nc.sync.dma_start(out=wala_sb[:, k, :], in_=wala[k * 128:(k + 1) * 128, :])
        wd1_sb = cpool.tile([128, 2, 128], bf)
        for k in range(2):
            nc.sync.dma_start(out=wd1_sb[:, k, :], in_=wd1[k * 128:(k + 1) * 128, :])
        wal1_sb = cpool.tile([128, 2, 128], bf)
        for k in range(2):
            nc.sync.dma_start(out=wal1_sb[:, k, :], in_=wal1[k * 128:(k + 1) * 128, :])
        wd2_sb = cpool.tile([128, 64], bf)
        nc.sync.dma_start(out=wd2_sb[:], in_=wd2[:])
        wd3_sb = cpool.tile([64, 3], bf)
        nc.sync.dma_start(out=wd3_sb[:], in_=wd3[:])
        bda_sb = cpool.tile([1, 256], bf)
        nc.sync.dma_start(out=bda_sb[:], in_=bda[:])
        bala_sb = cpool.tile([1, 256], bf)
        nc.sync.dma_start(out=bala_sb[:], in_=bala[:])
        bd1_sb = cpool.tile([1, 128], bf)
        nc.sync.dma_start(out=bd1_sb[:], in_=bd1[:])
        bal1_sb = cpool.tile([1, 128], bf)
        nc.sync.dma_start(out=bal1_sb[:], in_=bal1[:])
        bd2_sb = cpool.tile([1, 64], bf)
        nc.sync.dma_start(out=bd2_sb[:], in_=bd2[:])
        bd3_sb = cpool.tile([1, 3], bf)
        nc.sync.dma_start(out=bd3_sb[:], in_=bd3[:])

        yaT_sb = cpool.tile([5, ND], f32)      # rhs-side aug [1, sq, y]
        laT_sb = cpool.tile([5, ND], f32)      # lhsT-side aug [sq, 1, -2y]

        def layer_norm_affine(tc_ps, width, g_bc, b_bc, sm_pool):
            """Returns z_norm*g + b in SBUF (f32) from psum z [128, width]."""
            stats = sm_pool.tile([128, 6], f32, tag="stats")
            nc.vector.bn_stats(out=stats[:], in_=tc_ps[:])
            mv = sm_pool.tile([128, 2], f32, tag="mv")
            nc.vector.bn_aggr(out=mv[:], in_=stats[:])
            sd = sm_pool.tile([128, 1], f32, tag="sd")
            nc.scalar.activation(out=sd[:], in_=mv[:, 1:2], func=AF.Sqrt, bias=eps_sb[:, 0:1])
            rs = sm_pool.tile([128, 1], f32, tag="rs")
            nc.vector.reciprocal(out=rs[:], in_=sd[:])
            nmr = sm_pool.tile([128, 1], f32, tag="nmr")
            nc.vector.tensor_scalar(out=nmr[:], in0=mv[:, 0:1], scalar1=rs[:, 0:1],
                                    scalar2=-1.0, op0=ALU.mult, op1=ALU.mult)
            zn = sm_pool.tile([128, width], f32, tag="zn")
            nc.scalar.activation(out=zn[:], in_=tc_ps[:], func=AF.Identity,
                                 bias=nmr[:, 0:1], scale=rs[:, 0:1])
            zg = sm_pool.tile([128, width], f32, tag="zg")
            nc.vector.tensor_tensor(out=zg[:], in0=zn[:], in1=g_bc[:, 0:width], op=ALU.mult)
            zb = sm_pool.tile([128, width], f32, tag="zb")
            nc.vector.tensor_tensor(out=zb[:], in0=zg[:], in1=b_bc[:, 0:width], op=ALU.add)
            return zb

        with tc.tile_pool(name="mlpps", bufs=2, space="PSUM") as mps, \
             tc.tile_pool(name="mlpsm", bufs=2) as sm_pool, \
             tc.tile_pool(name="mlpsb", bufs=2) as msb:
            for b in range(NB):
                h1T = h1Ts[b]
                # layer a: za = h1@Wd_a + bd_a ; res = h1@Wal_a + bal_a
                za = mps.tile([128, 256], f32, tag="mm0")
                for k in range(4):
                    nc.tensor.matmul(out=za[:], lhsT=h1T[:, k, :], rhs=wda_sb[:, k, :],
                                     start=(k == 0), stop=False)
                nc.tensor.matmul(out=za[:], lhsT=ones1[:], rhs=bda_sb[:], start=False, stop=True)
                res = mps.tile([128, 256], f32, tag="mm1")
                for k in range(4):
                    nc.tensor.matmul(out=res[:], lhsT=h1T[:, k, :], rhs=wala_sb[:, k, :],
                                     start=(k == 0), stop=False)
                nc.tensor.matmul(out=res[:], lhsT=ones1[:], rhs=bala_sb[:], start=False, stop=True)
                zb = layer_norm_affine(za, 256, ga_bc, ba_bc, sm_pool)
                zr = msb.tile([128, 256], f32, tag="zr")
                nc.scalar.activation(out=zr[:], in_=zb[:], func=AF.Relu)
                h2 = msb.tile([128, 256], bf, tag="h2")
                nc.vector.tensor_tensor(out=h2[:], in0=zr[:], in1=res[:], op=ALU.add)
                h2T_ps = mps.tile([128, 256], bf, tag="tp")
                for k in range(2):
                    nc.tensor.transpose(out=h2T_ps[:, k * 128:(k + 1) * 128],
                                        in_=h2[:, k * 128:(k + 1) * 128], identity=ident[:])
                h2T = msb.tile([128, 2, 128], bf, tag="h2Ts")
                nc.vector.tensor_copy(out=h2T[:], in_=h2T_ps[:])
                # layer 1
                z1 = mps.tile([128, 128], f32, tag="mm0")
                for k in range(2):
                    nc.tensor.matmul(out=z1[:], lhsT=h2T[:, k, :], rhs=wd1_sb[:, k, :],
                                     start=(k == 0), stop=False)
                nc.tensor.matmul(out=z1[:], lhsT=ones1[:], rhs=bd1_sb[:], start=False, stop=True)
                res1 = mps.tile([128, 128], f32, tag="mm1")
                for k in range(2):
                    nc.tensor.matmul(out=res1[:], lhsT=h2T[:, k, :], rhs=wal1_sb[:, k, :],
                                     start=(k == 0), stop=False)
                nc.tensor.matmul(out=res1[:], lhsT=ones1[:], rhs=bal1_sb[:], start=False, stop=True)
                zb1 = layer_norm_affine(z1, 128, g1_bc, b1_bc, sm_pool)
                zr1 = msb.tile([128, 128], f32, tag="zr1")
                nc.scalar.activation(out=zr1[:], in_=zb1[:], func=AF.Relu)
                h3 = msb.tile([128, 128], bf, tag="h3")
                nc.vector.tensor_tensor(out=h3[:], in0=zr1[:], in1=res1[:], op=ALU.add)
                h3T_ps = mps.tile([128, 128], bf, tag="tp")
                nc.tensor.transpose(out=h3T_ps[:], in_=h3[:], identity=ident[:])
                h3T = msb.tile([128, 128], bf, tag="h3Ts")
                nc.vector.tensor_copy(out=h3T[:], in_=h3T_ps[:])
                # layer 2 (no residual)
                z2 = mps.tile([128, 64], f32, tag="mm0")
                nc.tensor.matmul(out=z2[:], lhsT=h3T[:], rhs=wd2_sb[:], start=True, stop=False)
                nc.tensor.matmul(out=z2[:], lhsT=ones1[:], rhs=bd2_sb[:], start=False, stop=True)
                zb2 = layer_norm_affine(z2, 64, g2_bc, b2_bc, sm_pool)
                h4 = msb.tile([128, 64], bf, tag="h4")
                nc.scalar.activation(out=h4[:], in_=zb2[:], func=AF.Relu)
                h4T_ps = mps.tile([64, 128], bf, tag="tp")
                nc.tensor.transpose(out=h4T_ps[:], in_=h4[:, :64], identity=ident[:])
                h4T = msb.tile([64, 128], bf, tag="h4Ts")
                nc.vector.tensor_copy(out=h4T[:], in_=h4T_ps[:])
                # y = h4 @ Wd3 + bd3
                y_ps = mps.tile([128, 3], f32, tag="mm0")
                nc.tensor.matmul(out=y_ps[:], lhsT=h4T[:], rhs=wd3_sb[:], start=True, stop=False)
                nc.tensor.matmul(out=y_ps[:], lhsT=ones1[:], rhs=bd3_sb[:], start=False, stop=True)
                # sq = sum(y^2), aug vectors
                ysq = msb.tile([128, 3], f32, tag="ysq")
                sq = msb.tile([128, 1], f32, tag="sq")
                nc.scalar.activation(out=ysq[:], in_=y_ps[:], func=AF.Square, accum_out=sq[:])
                ya = msb.tile([128, 5], f32, tag="ya")       # [1, sq, y]
                nc.vector.memset(ya[:, 0:1], 1.0)
                nc.vector.tensor_copy(out=ya[:, 1:2], in_=sq[:])
                nc.vector.tensor_copy(out=ya[:, 2:5], in_=y_ps[:])
                la = msb.tile([128, 5], f32, tag="la")       # [sq, 1, -2y]
                nc.vector.tensor_copy(out=la[:, 0:1], in_=sq[:])
                nc.vector.memset(la[:, 1:2], 1.0)
                nc.scalar.activation(out=la[:, 2:5], in_=y_ps[:], func=AF.Copy, scale=-2.0)
                yaT_ps = mps.tile([5, 128], f32, tag="tp")
                nc.tensor.transpose(out=yaT_ps[:], in_=ya[:], identity=identf[:])
                nc.vector.tensor_copy(out=yaT_sb[:, b * 128:(b + 1) * 128], in_=yaT_ps[:])
                laT_ps = mps.tile([5, 128], f32, tag="tp")
                nc.tensor.transpose(out=laT_ps[:], in_=la[:], identity=identf[:])
                nc.vector.tensor_copy(out=laT_sb[:, b * 128:(b + 1) * 128], in_=laT_ps[:])

        # ---------- AllGather y augmentation ----------
        nc.sync.dma_start(out=cc_in[:], in_=yaT_sb[:])
        nc.gpsimd.collective_compute(
            "AllGather", ALU.bypass,
            replica_groups=[list(range(NC))],
            ins=[cc_in[:].opt()],
            outs=[cc_out[:].opt()],
        )
        ag_tiles = []
        for c in range(NC):
            agt = cpool.tile([5, ND], f32, name=f"ag{c}")
            nc.sync.dma_start(out=agt[:], in_=cc_out[5 * c:5 * (c + 1), :])
            ag_tiles.append(agt)

        # ---------- Phase D: cdist ----------
        CH = N // 512   # 24 column chunks of 512
        with tc.tile_pool(name="dps", bufs=4, space="PSUM") as dps, \
             tc.tile_pool(name="dsb", bufs=4) as dsb:
            for m in range(NB):
                for ch in range(CH):
                    cidx = ch * 512
                    cc = cidx % ND
                    d2 = dps.tile([128, 512], f32, tag="d2")
                    nc.tensor.matmul(out=d2[:], lhsT=laT_sb[:, m * 128:(m + 1) * 128],
                                     rhs=ag_tiles[cidx // ND][:, cc:cc + 512],
                                     start=True, stop=True)
                    dr = dsb.tile([128, 512], f32, tag="dr")
                    nc.vector.tensor_scalar(out=dr[:], in0=d2[:], scalar1=0.0,
                                            scalar2=None, op0=ALU.max)
                    dsq = dsb.tile([128, 512], f32, tag="dsq")
                    nc.scalar.activation(out=dsq[:], in_=dr[:], func=AF.Sqrt)
                    nc.sync.dma_start(out=out[m * 128:(m + 1) * 128, cidx:cidx + 512],
                                      in_=dsq[:])
    nc.compile()
    return nc


# ---------------- host preprocessing ----------------

def host_prep(inputs):
    bf16 = ml_dtypes.bfloat16
    x = np.asarray(inputs["x"], np.float32)
    ei = np.asarray(inputs["edge_index"])
    src = np.concatenate([ei[0], np.arange(N)]).astype(np.int32)
    dst = np.concatenate([ei[1], np.arange(N)]).astype(np.int32)
    order = np.argsort(dst, kind="stable")
    src, dst = src[order], dst[order]
    blk = dst // 128
    counts = np.bincount(blk, minlength=NC * NB)
    assert counts.max() <= TB * 128, f"block overflow: {counts.max()}"
    starts = np.zeros(NC * NB + 1, np.int64)
    np.cumsum(counts, out=starts[1:])

    xT_b = np.ascontiguousarray(x.T).astype(bf16)
    att = np.asarray(inputs["att"], np.float32)
    attT_np = np.zeros((F, 2), np.float32)
    attT_np[0:C, 0] = att[0]
    attT_np[C:F, 1] = att[1]

    def b16(name):
        return np.ascontiguousarray(np.asarray(inputs[name], np.float32)).astype(bf16)

    def row16(name):
        return np.asarray(inputs[name], np.float32)[None, :].astype(bf16)

    def rowf(name):
        return np.ascontiguousarray(np.asarray(inputs[name], np.float32)[None, :])

    shared = dict(
        xT=xT_b,
        wl=b16("Wl"), blr=row16("bl"),
        wr=b16("Wr"), brr=row16("br"),
        attT=attT_np.astype(bf16),
        biasc=row16("bias_c"),
        wda=b16("Wd_a"), bda=row16("bd_a"),
        wala=b16("Wal_a"), bala=row16("bal_a"),
        wd1=b16("Wd1"), bd1=row16("bd1"),
        wal1=b16("Wal1"), bal1=row16("bal1"),
        wd2=b16("Wd2"), bd2=row16("bd2"),
        wd3=b16("Wd3"), bd3=row16("bd3"),
        lnga=rowf("ga"), lnba=rowf("bta"),
        lng1=rowf("g1"), lnb1=rowf("bt1"),
        lng2=rowf("g2"), lnb2=rowf("bt2"),
    )
    in_maps = []
    for c in range(NC):
        si = np.zeros((NB, 128, TB), np.int32)
        mt = np.zeros((NB, 128, TB, 128), bf16)
        m01 = np.zeros((NB, 128, TB, 128), bf16)
        for b in range(NB):
            g = c * NB + b
            s, e = starts[g], starts[g + 1]
            cnt = e - s
            esrc = src[s:e]
            edl = (dst[s:e] - (g * 128)).astype(np.int32)
            pad = TB * 128 - cnt
            esrc = np.concatenate([esrc, np.zeros(pad, np.int32)])
            edl = np.concatenate([edl, np.full(pad, 255, np.int32)])
            esrc = esrc.reshape(TB, 128)
            edl = edl.reshape(TB, 128)
            si[b] = esrc.T
            onehot = (edl[:, :, None] == np.arange(128)[None, None, :])
            mt[b] = onehot.transpose(1, 0, 2).astype(bf16)
            m01[b] = onehot.transpose(2, 0, 1).astype(bf16)
        m = dict(shared)
        m["xTl"] = np.ascontiguousarray(xT_b[:, c * ND:(c + 1) * ND])
        m["sidx"] = si
        m["mt_in"] = mt
        m["m01_in"] = m01
        in_maps.append(m)
    return in_maps


class _Runner:
    """Compile once; keep a reusable jitted sharded executable.

    Mirrors concourse.bass2jax.run_bass_via_pjrt's multi-core path, but
    memoizes the jit so repeated calls don't re-trace, and exposes a
    device-resident timing mode.
    """

    def __init__(self):
        import jax
        import concourse.mybir as mb
        from concourse import bass2jax
        from jax.sharding import Mesh, PartitionSpec, NamedSharding
        from jax.experimental.shard_map import shard_map

        bass2jax.install_neuronx_cc_hook()
        nc = build_program()
        self.nc = nc
        part_name = nc.partition_id_tensor.name if nc.partition_id_tensor else None
        in_names, out_names, out_avals, zero_shapes = [], [], [], []
        for alloc in nc.m.functions[0].allocations:
            if not isinstance(alloc, mb.MemoryLocationSet):
                continue
            name = alloc.memorylocations[0].name
            if alloc.kind == "ExternalInput":
                if name != part_name:
                    in_names.append(name)
            elif alloc.kind == "ExternalOutput":
                out_names.append(name)
                out_avals.append(jax.core.ShapedArray(
                    tuple(alloc.tensor_shape), mb.dt.np(alloc.dtype)))
                zero_shapes.append((tuple(alloc.tensor_shape), mb.dt.np(alloc.dtype)))
        n_params = len(in_names)
        n_outs = len(out_names)
        all_names = in_names + out_names
        if part_name is not None:
            all_names = all_names + [part_name]
        self.in_names = in_names
        self.out_names = out_names
        donate = tuple(range(n_params, n_params + n_outs))

        def _body(*args):
            operands = list(args)
            if part_name is not None:
                operands.append(bass2jax.partition_id_tensor())
            outs = bass2jax._bass_exec_p.bind(
                *operands,
                out_avals=tuple(out_avals),
                in_names=tuple(all_names),
                out_names=tuple(out_names),
                lowering_input_output_aliases=(),
                sim_require_finite=True,
                sim_require_nnan=True,
                nc=nc,
            )
            return tuple(outs)

        devices = jax.devices()[:NC]
        mesh = Mesh(np.asarray(devices), ("core",))
        self.mesh = mesh
        self.sharding = NamedSharding(mesh, PartitionSpec("core"))
        in_specs = (PartitionSpec("core"),) * (n_params + n_outs)
        out_specs = (PartitionSpec("core"),) * n_outs
        self.sharded = jax.jit(
            shard_map(_body, mesh=mesh, in_specs=in_specs, out_specs=out_specs,
                      check_rep=False),
            donate_argnums=donate, keep_unused=True)

        import jax.numpy as jnp

        def _zeros():
            return tuple(jnp.zeros((NC * s[0], *s[1:]), d) for s, d in zero_shapes)

        self.zmaker = jax.jit(_zeros, out_shardings=(self.sharding,) * n_outs)
        self.jax = jax

    def put_inputs(self, in_maps):
        concat = [np.concatenate([np.asarray(in_maps[c][k]) for c in range(NC)], axis=0)
                  for k in self.in_names]
        return [self.jax.device_put(a, self.sharding) for a in concat]

    def exec_dev(self, dev_inputs):
        zs = self.zmaker()
        outs = self.sharded(*dev_inputs, *zs)
        self.jax.block_until_ready(outs)
        return outs

    def run(self, in_maps):
        dev_inputs = self.put_inputs(in_maps)
        outs = self.exec_dev(dev_inputs)
        res = []
        for c in range(NC):
            d = {}
            for i, name in enumerate(self.out_names):
                arr = np.asarray(outs[i])
                d[name] = arr.reshape(NC, -1, *arr.shape[1:])[c].reshape(
                    arr.shape[0] // NC, *arr.shape[1:])
            res.append(d)
        return res


def _get_runner():
    if "runner" not in _cache:
        _cache["runner"] = _Runner()
    return _cache["runner"]


def kernel(**inputs):
    runner = _get_runner()
    in_maps = host_prep(inputs)
    results = runner.run(in_maps)
    out = np.concatenate([results[c]["out"] for c in range(NC)], axis=0)
    return out


# revision 9
# speedup vs baseline: 1.1069x; 1.1069x over previous
"""Self-contained Trainium2 kernel for nn_GATNetSelectiveResidualsUpdated.

GATv2 layer + MLP head + pairwise-distance output, distributed over 8
NeuronCores: dst-nodes (and the cdist row block) are sharded per core,
edges grouped by dst block, xl table built redundantly per core, y
all-gathered on-device for the cdist columns.

kernel(**inputs) takes the FULL inputs (as produced by setup_inputs())
and returns the FULL [12288, 12288] float32 output.
"""
import numpy as np
import ml_dtypes
from contextlib import ExitStack

import concourse.bass as bass
import concourse.bacc as bacc
import concourse.mybir as mybir
from concourse import tile
from concourse.masks import make_identity

dt = mybir.dt
AF = mybir.ActivationFunctionType
ALU = mybir.AluOpType

N = 12288
F = 512          # H*C
C = 256
NC = 8
ND = N // NC     # 1536 nodes per core
NB = ND // 128   # 12 dst blocks per core
TB = 27          # edge tiles per block (padded)
NM = N // 128    # 96 m-tiles for full table
EPS = 1e-5

bf = dt.float16
f32 = dt.float32

_cache = {}


def build_program():
    nc = bacc.Bacc(None, target_bir_lowering=False)
    xT = nc.dram_tensor("xT", [F, N], bf, kind="ExternalInput")
    xTl = nc.dram_tensor("xTl", [F, ND], bf, kind="ExternalInput")
    wl = nc.dram_tensor("wl", [F, F], bf, kind="ExternalInput")
    blr = nc.dram_tensor("blr", [1, F], bf, kind="ExternalInput")
    wr = nc.dram_tensor("wr", [F, F], bf, kind="ExternalInput")
    brr = nc.dram_tensor("brr", [1, F], bf, kind="ExternalInput")
    attT = nc.dram_tensor("attT", [F, 2], bf, kind="ExternalInput")
    biasc = nc.dram_tensor("biasc", [1, F], bf, kind="ExternalInput")
    sidx = nc.dram_tensor("sidx", [NB, 128, TB], dt.int32, kind="ExternalInput")
    mt_in = nc.dram_tensor("mt_in", [NB, 128, TB, 128], bf, kind="ExternalInput")
    m01_in = nc.dram_tensor("m01_in", [NB, 128, TB, 128], bf, kind="ExternalInput")
    # MLP weights
    wda = nc.dram_tensor("wda", [F, 256], bf, kind="ExternalInput")
    bda = nc.dram_tensor("bda", [1, 256], bf, kind="ExternalInput")
    wala = nc.dram_tensor("wala", [F, 256], bf, kind="ExternalInput")
    bala = nc.dram_tensor("bala", [1, 256], bf, kind="ExternalInput")
    wd1 = nc.dram_tensor("wd1", [256, 128], bf, kind="ExternalInput")
    bd1 = nc.dram_tensor("bd1", [1, 128], bf, kind="ExternalInput")
    wal1 = nc.dram_tensor("wal1", [256, 128], bf, kind="ExternalInput")
    bal1 = nc.dram_tensor("bal1", [1, 128], bf, kind="ExternalInput")
    wd2 = nc.dram_tensor("wd2", [128, 64], bf, kind="ExternalInput")
    bd2 = nc.dram_tensor("bd2", [1, 64], bf, kind="ExternalInput")
    wd3 = nc.dram_tensor("wd3", [64, 3], bf, kind="ExternalInput")
    bd3 = nc.dram_tensor("bd3", [1, 3], bf, kind="ExternalInput")
    # LN affine params (f32)
    lnga = nc.dram_tensor("lnga", [1, 256], f32, kind="ExternalInput")
    lnba = nc.dram_tensor("lnba", [1, 256], f32, kind="ExternalInput")
    lng1 = nc.dram_tensor("lng1", [1, 128], f32, kind="ExternalInput")
    lnb1 = nc.dram_tensor("lnb1", [1, 128], f32, kind="ExternalInput")
    lng2 = nc.dram_tensor("lng2", [1, 64], f32, kind="ExternalInput")
    lnb2 = nc.dram_tensor("lnb2", [1, 64], f32, kind="ExternalInput")

    table = nc.dram_tensor("table", [N, 514], bf, kind="Internal")
    cc_in = nc.dram_tensor("cc_in", [5, ND], f32, kind="Internal")
    cc_out = nc.dram_tensor("cc_out", [NC * 5, ND], f32, kind="Internal",
                            addr_space="Shared")
    out = nc.dram_tensor("out", [ND, N], f32, kind="ExternalOutput")
    y_out = nc.dram_tensor("y_out", [ND, 3], f32, kind="ExternalOutput")

    with tile.TileContext(nc) as tc, ExitStack() as ctx:
        cpool = ctx.enter_context(tc.tile_pool(name="const", bufs=1))
        ident = cpool.tile([128, 128], bf)
        make_identity(nc, ident[:])
        identf = cpool.tile([128, 128], f32)
        make_identity(nc, identf[:])
        ones1 = cpool.tile([1, 128], bf)
        nc.vector.memset(ones1[:], 1.0)
        eps_sb = cpool.tile([128, 1], f32)
        nc.vector.memset(eps_sb[:], EPS)
        att_sb = cpool.tile([128, 4, 2], bf)
        nc.sync.dma_start(out=att_sb[:], in_=attT[:].rearrange("(a p) h -> p a h", p=128))
        biasc_sb = cpool.tile([1, F], bf)
        nc.sync.dma_start(out=biasc_sb[:], in_=biasc[:])

        # broadcast constants to all partitions via rank-1 matmuls
        def bcast_f32(row_dram, width, name):
            row_sb = cpool.tile([1, width], f32, name=f"{name}_row")
            nc.sync.dma_start(out=row_sb[:], in_=row_dram[:])
            row_bf = cpool.tile([1, width], bf, name=f"{name}_bf")
            nc.vector.tensor_copy(out=row_bf[:], in_=row_sb[:])
            ps = initps.tile([128, width], f32, tag="initps")
            nc.tensor.matmul(out=ps[:], lhsT=ones1[:], rhs=row_bf[:], start=True, stop=True)
            bcast = cpool.tile([128, width], f32, name=f"{name}_bc")
            nc.vector.tensor_copy(out=bcast[:], in_=ps[:])
            # exact affine correction: bcast = bcast - bf16err  (bf16 round-trip
            # loses precision; add the residual from the f32 row)
            return bcast, row_sb

        with tc.tile_pool(name="initps", bufs=2, space="PSUM") as initps:
            ps = initps.tile([128, F], f32, tag="initps")
            nc.tensor.matmul(out=ps[:], lhsT=ones1[:], rhs=biasc_sb[:], start=True, stop=True)
            biasc_bc = cpool.tile([128, F], f32)
            nc.vector.tensor_copy(out=biasc_bc[:], in_=ps[:])
            ga_bc, _ = bcast_f32(lnga, 256, "ga")
            ba_bc, _ = bcast_f32(lnba, 256, "ba")
            g1_bc, _ = bcast_f32(lng1, 128, "g1")
            b1_bc, _ = bcast_f32(lnb1, 128, "b1")
            g2_bc, _ = bcast_f32(lng2, 64, "g2")
            b2_bc, _ = bcast_f32(lnb2, 64, "b2")

        # ---------- Phase A1: full xl table ----------
        wl_sb = cpool.tile([128, 4, F], bf)
        for k in range(4):
            nc.sync.dma_start(out=wl_sb[:, k, :], in_=wl[k * 128:(k + 1) * 128, :])
        bl_sb = cpool.tile([1, F], bf)
        nc.sync.dma_start(out=bl_sb[:], in_=blr[:])
        wr_sb = cpool.tile([128, 4, F], bf)
        for k in range(4):
            nc.sync.dma_start(out=wr_sb[:, k, :], in_=wr[k * 128:(k + 1) * 128, :])
        br_sb = cpool.tile([1, F], bf)
        nc.sync.dma_start(out=br_sb[:], in_=brr[:])

        xT_v = xT[:].rearrange("(a p) n -> p a n", p=128)
        with tc.tile_pool(name="a1x", bufs=3) as xa_pool, \
             tc.tile_pool(name="a1ps", bufs=2, space="PSUM") as ps_a, \
             tc.tile_pool(name="a1bld", bufs=3) as bld_pool:
            for m in range(NM):
                xa = xa_pool.tile([128, 4, 128], bf)
                nc.sync.dma_start(out=xa[:], in_=xT_v[:, :, m * 128:(m + 1) * 128])
                ps = ps_a.tile([128, F], f32)
                for k in range(4):
                    nc.tensor.matmul(out=ps[:], lhsT=xa[:, k, :], rhs=wl_sb[:, k, :],
                                     start=(k == 0), stop=False)
                nc.tensor.matmul(out=ps[:], lhsT=ones1[:], rhs=bl_sb[:], start=False, stop=True)
                bld = bld_pool.tile([128, 514], bf)
                nc.vector.memset(bld[:, 256:257], 1.0)
                nc.vector.memset(bld[:, 513:514], 1.0)
                nc.scalar.activation(out=bld[:, 0:256], in_=ps[:, 0:256], func=AF.Copy)
                nc.scalar.activation(out=bld[:, 257:513], in_=ps[:, 256:512], func=AF.Copy)
                nc.sync.dma_start(out=table[m * 128:(m + 1) * 128, :], in_=bld[:])

        # ---------- Phase A2: local xr (node-major, SBUF-resident) ----------
        vb_pool = ctx.enter_context(tc.tile_pool(name="vb", bufs=NB))
        vbs = []
        xTl_v = xTl[:].rearrange("(a p) n -> p a n", p=128)
        with tc.tile_pool(name="a2x", bufs=2) as xa2_pool, \
             tc.tile_pool(name="a2ps", bufs=2, space="PSUM") as ps_a2:
            for b in range(NB):
                xa = xa2_pool.tile([128, 4, 128], bf)
                nc.sync.dma_start(out=xa[:], in_=xTl_v[:, :, b * 128:(b + 1) * 128])
                ps = ps_a2.tile([128, F], f32)
                for k in range(4):
                    nc.tensor.matmul(out=ps[:], lhsT=xa[:, k, :], rhs=wr_sb[:, k, :],
                                     start=(k == 0), stop=False)
                nc.tensor.matmul(out=ps[:], lhsT=ones1[:], rhs=br_sb[:], start=False, stop=True)
                vb = vb_pool.tile([128, F], bf, tag="vb", bufs=NB)
                nc.scalar.activation(out=vb[:], in_=ps[:], func=AF.Copy)
                vbs.append(vb)

        # h1T storage (transposed GAT output, bf16, persistent)
        h1T_pool = ctx.enter_context(tc.tile_pool(name="h1T", bufs=NB))
        h1Ts = []

        # ---------- Phase B: edge loop ----------
        with tc.tile_pool(name="mtp", bufs=2) as mt_pool, \
             tc.tile_pool(name="m01p", bufs=2) as m01_pool, \
             tc.tile_pool(name="sip", bufs=2) as si_pool, \
             tc.tile_pool(name="xgp", bufs=3) as xg_pool, \
             tc.tile_pool(name="pmp", bufs=2, space="PSUM") as pm_pool, \
             tc.tile_pool(name="tpsp", bufs=2, space="PSUM") as tps_pool, \
             tc.tile_pool(name="esp", bufs=2, space="PSUM") as es_ps, \
             tc.tile_pool(name="Up", bufs=2, space="PSUM") as U_pool, \
             tc.tile_pool(name="rsbp", bufs=2) as rsb_pool, \
             tc.tile_pool(name="rTp", bufs=2) as rT_pool, \
             tc.tile_pool(name="essb", bufs=2) as essb_pool, \
             tc.tile_pool(name="szp", bufs=2) as sz_pool, \
             tc.tile_pool(name="h1p", bufs=2) as h1_pool:
            for b in range(NB):
                mt_sb = mt_pool.tile([128, TB, 128], bf)
                nc.sync.dma_start(out=mt_sb[:], in_=mt_in[b])
                m01_sb = m01_pool.tile([128, TB, 128], bf)
                nc.sync.dma_start(out=m01_sb[:], in_=m01_in[b])
                si_sb = si_pool.tile([128, TB], dt.int32)
                nc.sync.dma_start(out=si_sb[:], in_=sidx[b])
                U0 = U_pool.tile([128, 257], f32, tag="U")
                U1 = U_pool.tile([128, 257], f32, tag="U")
                for t in range(TB):
                    xg = xg_pool.tile([128, 514], bf)
                    nc.gpsimd.indirect_dma_start(
                        out=xg[:], out_offset=None, in_=table[:],
                        in_offset=bass.IndirectOffsetOnAxis(ap=si_sb[:, t:t + 1], axis=0))
                    pm = pm_pool.tile([128, F], f32)
                    xg_m = xg[:].rearrange("p (h x) -> p h x", x=257)[:, :, 0:256]
                    nc.tensor.matmul(out=pm[:], lhsT=ident[:], rhs=xg_m, start=True, stop=False)
                    nc.tensor.matmul(out=pm[:], lhsT=m01_sb[:, t, :], rhs=vbs[b][:],
                                     start=False, stop=True)
                    r_sb = rsb_pool.tile([128, F], bf)
                    nc.scalar.activation(out=r_sb[:], in_=pm[:], func=AF.Prelu, alpha=0.2)
                    rT_ps = tps_pool.tile([128, F], bf)
                    for k in range(4):
                        nc.tensor.transpose(out=rT_ps[:, k * 128:(k + 1) * 128],
                                            in_=r_sb[:, k * 128:(k + 1) * 128],
                                            identity=ident[:])
                    rT_sb = rT_pool.tile([128, F], bf)
                    nc.vector.tensor_copy(out=rT_sb[:], in_=rT_ps[:])
                    eT = es_ps.tile([2, 128], f32, tag="es")
                    for k in range(4):
                        nc.tensor.matmul(out=eT[:], lhsT=att_sb[:, k, :],
                                         rhs=rT_sb[:, k * 128:(k + 1) * 128],
                                         start=(k == 0), stop=(k == 3))
                    zT = essb_pool.tile([2, 128], bf, tag="zT")
                    nc.scalar.activation(out=zT[:], in_=eT[:], func=AF.Exp)
                    zps = es_ps.tile([128, 2], bf, tag="es")
                    nc.tensor.transpose(out=zps[:], in_=zT[:], identity=ident[:2, :2])
                    z_sb = essb_pool.tile([128, 2], f32, tag="zs")
                    nc.vector.tensor_copy(out=z_sb[:], in_=zps[:])
                    sz = sz_pool.tile([128, 2, 128], bf)
                    nc.vector.tensor_scalar(out=sz[:, 0, :], in0=mt_sb[:, t, :],
                                            scalar1=z_sb[:, 0:1], scalar2=None, op0=ALU.mult)
                    nc.vector.tensor_scalar(out=sz[:, 1, :], in0=mt_sb[:, t, :],
                                            scalar1=z_sb[:, 1:2], scalar2=None, op0=ALU.mult)
                    nc.tensor.matmul(out=U0[:], lhsT=sz[:, 0, :], rhs=xg[:, 0:257],
                                     start=(t == 0), stop=(t == TB - 1))
                    nc.tensor.matmul(out=U1[:], lhsT=sz[:, 1, :], rhs=xg[:, 257:514],
                                     start=(t == 0), stop=(t == TB - 1))
                # h1 assembly for block b
                rc = essb_pool.tile([128, 2], f32, tag="rc")
                nc.vector.reciprocal(out=rc[:, 0:1], in_=U0[:, 256:257])
                nc.vector.reciprocal(out=rc[:, 1:2], in_=U1[:, 256:257])
                h1a = h1_pool.tile([128, F], f32, tag="h1a")
                nc.vector.tensor_scalar(out=h1a[:, 0:256], in0=U0[:, 0:256],
                                        scalar1=rc[:, 0:1], scalar2=None, op0=ALU.mult)
                nc.vector.tensor_scalar(out=h1a[:, 256:512], in0=U1[:, 0:256],
                                        scalar1=rc[:, 1:2], scalar2=None, op0=ALU.mult)
                h1b = h1_pool.tile([128, F], f32, tag="h1b")
                nc.vector.tensor_tensor(out=h1b[:], in0=h1a[:], in1=biasc_bc[:], op=ALU.add)
                h1f = h1_pool.tile([128, F], bf, tag="h1f")
                nc.scalar.activation(out=h1f[:], in_=h1b[:], func=AF.Relu)
                h1T_ps = tps_pool.tile([128, F], bf, tag="rT_ps")
                for k in range(4):
                    nc.tensor.transpose(out=h1T_ps[:, k * 128:(k + 1) * 128],
                                        in_=h1f[:, k * 128:(k + 1) * 128],
                                        identity=ident[:])
                h1T = h1T_pool.tile([128, 4, 128], bf, tag="h1T", bufs=NB)
                nc.vector.tensor_copy(out=h1T[:], in_=h1T_ps[:])
                h1Ts.append(h1T)

        # ---------- Phase C: MLP head ----------
        # load MLP weights
        wda_sb = cpool.tile([128, 4, 256], bf)
        for k in range(4):
            nc.sync.dma_start(out=wda_sb[:, k, :], in_=wda[k * 128:(k + 1) * 128, :])
        wala_sb = cpool.tile([128, 4, 256], bf)
        for k in range(4):
            nc.sync.dma_start(out=wala_sb[:, k, :], in_=wala[k * 128:(k + 1) * 128, :])
        wd1_sb = cpool.tile([128, 2, 128], bf)
        for k in range(2):
            nc.sync.dma_start(out=wd1_sb[:, k, :], in_=wd1[k * 128:(k + 1) * 128, :])
        wal1_sb = cpool.tile([128, 2, 128], bf)
        for k in range(2):
            nc.sync.dma_start(out=wal1_sb[:, k, :], in_=wal1[k * 128:(k + 1) * 128, :])
        wd2_sb = cpool.tile([128, 64], bf)
        nc.sync.dma_start(out=wd2_sb[:], in_=wd2[:])
        wd3_sb = cpool.tile([64, 3], bf)
        nc.sync.dma_start(out=wd3_sb[:], in_=wd3[:])
        bda_sb = cpool.tile([1, 256], bf)
        nc.sync.dma_start(out=bda_sb[:], in_=bda[:])
        bala_sb = cpool.tile([1, 256], bf)
        nc.sync.dma_start(out=bala_sb[:], in_=bala[:])
        bd1_sb = cpool.tile([1, 128], bf)
        nc.sync.dma_start(out=bd1_sb[:], in_=bd1[:])
        bal1_sb = cpool.tile([1, 128], bf)
        nc.sync.dma_start(out=bal1_sb[:], in_=bal1[:])
        bd2_sb = cpool.tile([1, 64], bf)
        nc.sync.dma_start(out=bd2_sb[:], in_=bd2[:])
        bd3_sb = cpool.tile([1, 3], bf)
        nc.sync.dma_start(out=bd3_sb[:], in_=bd3[:])

        yaT_sb = cpool.tile([5, ND], f32)      # rhs-side aug [1, sq, y]
        laT_sb = cpool.tile([5, ND], f32)      # lhsT-side aug [sq, 1, -2y]

        def layer_norm_affine(tc_ps, width, g_bc, b_bc, sm_pool):
            """Returns z_norm*g + b in SBUF (f32) from psum z [128, width]."""
            stats = sm_pool.tile([128, 6], f32, tag="stats")
            nc.vector.bn_stats(out=stats[:], in_=tc_ps[:])
            mv = sm_pool.tile([128, 2], f32, tag="mv")
            nc.vector.bn_aggr(out=mv[:], in_=stats[:])
            sd = sm_pool.tile([128, 1], f32, tag="sd")
            nc.scalar.activation(out=sd[:], in_=mv[:, 1:2], func=AF.Sqrt, bias=eps_sb[:, 0:1])
            rs = sm_pool.tile([128, 1], f32, tag="rs")
            nc.vector.reciprocal(out=rs[:], in_=sd[:])
            nmr = sm_pool.tile([128, 1], f32, tag="nmr")
            nc.vector.tensor_scalar(out=nmr[:], in0=mv[:, 0:1], scalar1=rs[:, 0:1],
                                    scalar2=-1.0, op0=ALU.mult, op1=ALU.mult)
            zn = sm_pool.tile([128, width], f32, tag="zn")
            nc.scalar.activation(out=zn[:], in_=tc_ps[:], func=AF.Identity,
                                 bias=nmr[:, 0:1], scale=rs[:, 0:1])
            zg = sm_pool.tile([128, width], f32, tag="zg")
            nc.vector.tensor_tensor(out=zg[:], in0=zn[:], in1=g_bc[:, 0:width], op=ALU.mult)
            zb = sm_pool.tile([128, width], f32, tag="zb")
            nc.vector.tensor_tensor(out=zb[:], in0=zg[:], in1=b_bc[:, 0:width], op=ALU.add)
            return zb

        with tc.tile_pool(name="mlpps", bufs=2, space="PSUM") as mps, \
             tc.tile_pool(name="mlpsm", bufs=2) as sm_pool, \
             tc.tile_pool(name="mlpsb", bufs=2) as msb:
            for b in range(NB):
                h1T = h1Ts[b]
                # layer a: za = h1@Wd_a + bd_a ; res = h1@Wal_a + bal_a
                za = mps.tile([128, 256], f32, tag="mm0")
                for k in range(4):
                    nc.tensor.matmul(out=za[:], lhsT=h1T[:, k, :], rhs=wda_sb[:, k, :],
                                     start=(k == 0), stop=False)
                nc.tensor.matmul(out=za[:], lhsT=ones1[:], rhs=bda_sb[:], start=False, stop=True)
                res = mps.tile([128, 256], f32, tag="mm1")
                for k in range(4):
                    nc.tensor.matmul(out=res[:], lhsT=h1T[:, k, :], rhs=wala_sb[:, k, :],
                                     start=(k == 0), stop=False)
                nc.tensor.matmul(out=res[:], lhsT=ones1[:], rhs=bala_sb[:], start=False, stop=True)
                zb = layer_norm_affine(za, 256, ga_bc, ba_bc, sm_pool)
                zr = msb.tile([128, 256], f32, tag="zr")
                nc.scalar.activation(out=zr[:], in_=zb[:], func=AF.Relu)
                h2 = msb.tile([128, 256], bf, tag="h2")
                nc.vector.tensor_tensor(out=h2[:], in0=zr[:], in1=res[:], op=ALU.add)
                h2T_ps = mps.tile([128, 256], bf, tag="tp")
                for k in range(2):
                    nc.tensor.transpose(out=h2T_ps[:, k * 128:(k + 1) * 128],
                                        in_=h2[:, k * 128:(k + 1) * 128], identity=ident[:])
                h2T = msb.tile([128, 2, 128], bf, tag="h2Ts")
                nc.vector.tensor_copy(out=h2T[:], in_=h2T_ps[:])
                # layer 1
                z1 = mps.tile([128, 128], f32, tag="mm0")
                for k in range(2):
                    nc.tensor.matmul(out=z1[:], lhsT=h2T[:, k, :], rhs=wd1_sb[:, k, :],
                                     start=(k == 0), stop=False)
                nc.tensor.matmul(out=z1[:], lhsT=ones1[:], rhs=bd1_sb[:], start=False, stop=True)
                res1 = mps.tile([128, 128], f32, tag="mm1")
                for k in range(2):
                    nc.tensor.matmul(out=res1[:], lhsT=h2T[:, k, :], rhs=wal1_sb[:, k, :],
                                     start=(k == 0), stop=False)
                nc.tensor.matmul(out=res1[:], lhsT=ones1[:], rhs=bal1_sb[:], start=False, stop=True)
                zb1 = layer_norm_affine(z1, 128, g1_bc, b1_bc, sm_pool)
                zr1 = msb.tile([128, 128], f32, tag="zr1")
                nc.scalar.activation(out=zr1[:], in_=zb1[:], func=AF.Relu)
                h3 = msb.tile([128, 128], bf, tag="h3")
                nc.vector.tensor_tensor(out=h3[:], in0=zr1[:], in1=res1[:], op=ALU.add)
                h3T_ps = mps.tile([128, 128], bf, tag="tp")
                nc.tensor.transpose(out=h3T_ps[:], in_=h3[:], identity=ident[:])
                h3T = msb.tile([128, 128], bf, tag="h3Ts")
                nc.vector.tensor_copy(out=h3T[:], in_=h3T_ps[:])
                # layer 2 (no residual)
                z2 = mps.tile([128, 64], f32, tag="mm0")
                nc.tensor.matmul(out=z2[:], lhsT=h3T[:], rhs=wd2_sb[:], start=True, stop=False)
                nc.tensor.matmul(out=z2[:], lhsT=ones1[:], rhs=bd2_sb[:], start=False, stop=True)
                zb2 = layer_norm_affine(z2, 64, g2_bc, b2_bc, sm_pool)
                h4 = msb.tile([128, 64], bf, tag="h4")
                nc.scalar.activation(out=h4[:], in_=zb2[:], func=AF.Relu)
                h4T_ps = mps.tile([64, 128], bf, tag="tp")
                nc.tensor.transpose(out=h4T_ps[:], in_=h4[:, :64], identity=ident[:])
                h4T = msb.tile([64, 128], bf, tag="h4Ts")
                nc.vector.tensor_copy(out=h4T[:], in_=h4T_ps[:])
                # y = h4 @ Wd3 + bd3
                y_ps = mps.tile([128, 3], f32, tag="mm0")
                nc.tensor.matmul(out=y_ps[:], lhsT=h4T[:], rhs=wd3_sb[:], start=True, stop=False)
                nc.tensor.matmul(out=y_ps[:], lhsT=ones1[:], rhs=bd3_sb[:], start=False, stop=True)
                # sq = sum(y^2), aug vectors
                ysq = msb.tile([128, 3], f32, tag="ysq")
                sq = msb.tile([128, 1], f32, tag="sq")
                nc.scalar.activation(out=ysq[:], in_=y_ps[:], func=AF.Square, accum_out=sq[:])
                ya = msb.tile([128, 5], f32, tag="ya")       # [1, sq, y]
                nc.vector.memset(ya[:, 0:1], 1.0)
                nc.vector.tensor_copy(out=ya[:, 1:2], in_=sq[:])
                nc.vector.tensor_copy(out=ya[:, 2:5], in_=y_ps[:])
                nc.sync.dma_start(out=y_out[b * 128:(b + 1) * 128, :], in_=ya[:, 2:5])
                la = msb.tile([128, 5], f32, tag="la")       # [sq, 1, -2y]
                nc.vector.tensor_copy(out=la[:, 0:1], in_=sq[:])
                nc.vector.memset(la[:, 1:2], 1.0)
                nc.scalar.activation(out=la[:, 2:5], in_=y_ps[:], func=AF.Copy, scale=-2.0)
                yaT_ps = mps.tile([5, 128], f32, tag="tp")
                nc.tensor.transpose(out=yaT_ps[:], in_=ya[:], identity=identf[:])
                nc.vector.tensor_copy(out=yaT_sb[:, b * 128:(b + 1) * 128], in_=yaT_ps[:])
                laT_ps = mps.tile([5, 128], f32, tag="tp")
                nc.tensor.transpose(out=laT_ps[:], in_=la[:], identity=identf[:])
                nc.vector.tensor_copy(out=laT_sb[:, b * 128:(b + 1) * 128], in_=laT_ps[:])

        # ---------- AllGather y augmentation ----------
        nc.sync.dma_start(out=cc_in[:], in_=yaT_sb[:])
        nc.gpsimd.collective_compute(
            "AllGather", ALU.bypass,
            replica_groups=[list(range(NC))],
            ins=[cc_in[:].opt()],
            outs=[cc_out[:].opt()],
        )
        ag_tiles = []
        for c in range(NC):
            agt = cpool.tile([5, ND], f32, name=f"ag{c}")
            nc.sync.dma_start(out=agt[:], in_=cc_out[5 * c:5 * (c + 1), :])
            ag_tiles.append(agt)

        # ---------- Phase D: cdist ----------
        CH = N // 512   # 24 column chunks of 512
        with tc.tile_pool(name="dps", bufs=4, space="PSUM") as dps, \
             tc.tile_pool(name="dsb", bufs=4) as dsb:
            for m in range(NB):
                for ch in range(CH):
                    cidx = ch * 512
                    cc = cidx % ND
                    d2 = dps.tile([128, 512], f32, tag="d2")
                    nc.tensor.matmul(out=d2[:], lhsT=laT_sb[:, m * 128:(m + 1) * 128],
                                     rhs=ag_tiles[cidx // ND][:, cc:cc + 512],
                                     start=True, stop=True)
                    dr = dsb.tile([128, 512], f32, tag="dr")
                    nc.vector.tensor_scalar(out=dr[:], in0=d2[:], scalar1=0.0,
                                            scalar2=None, op0=ALU.max)
                    dsq = dsb.tile([128, 512], f32, tag="dsq")
                    nc.scalar.activation(out=dsq[:], in_=dr[:], func=AF.Sqrt)
                    nc.sync.dma_start(out=out[m * 128:(m + 1) * 128, cidx:cidx + 512],
                                      in_=dsq[:])
    nc.compile()
    return nc


# ---------------- host preprocessing ----------------

def host_prep(inputs):
    bf16 = np.float16
    x = np.asarray(inputs["x"], np.float32)
    ei = np.asarray(inputs["edge_index"])
    src = np.concatenate([ei[0], np.arange(N)]).astype(np.int32)
    dst = np.concatenate([ei[1], np.arange(N)]).astype(np.int32)
    order = np.argsort(dst, kind="stable")
    src, dst = src[order], dst[order]
    blk = dst // 128
    counts = np.bincount(blk, minlength=NC * NB)
    assert counts.max() <= TB * 128, f"block overflow: {counts.max()}"
    starts = np.zeros(NC * NB + 1, np.int64)
    np.cumsum(counts, out=starts[1:])

    xT_b = np.ascontiguousarray(x.T).astype(bf16)
    att = np.asarray(inputs["att"], np.float32)
    attT_np = np.zeros((F, 2), np.float32)
    attT_np[0:C, 0] = att[0]
    attT_np[C:F, 1] = att[1]

    def b16(name):
        return np.ascontiguousarray(np.asarray(inputs[name], np.float32)).astype(bf16)

    def row16(name):
        return np.asarray(inputs[name], np.float32)[None, :].astype(bf16)

    def rowf(name):
        return np.ascontiguousarray(np.asarray(inputs[name], np.float32)[None, :])

    shared = dict(
        xT=xT_b,
        wl=b16("Wl"), blr=row16("bl"),
        wr=b16("Wr"), brr=row16("br"),
        attT=attT_np.astype(bf16),
        biasc=row16("bias_c"),
        wda=b16("Wd_a"), bda=row16("bd_a"),
        wala=b16("Wal_a"), bala=row16("bal_a"),
        wd1=b16("Wd1"), bd1=row16("bd1"),
        wal1=b16("Wal1"), bal1=row16("bal1"),
        wd2=b16("Wd2"), bd2=row16("bd2"),
        wd3=b16("Wd3"), bd3=row16("bd3"),
        lnga=rowf("ga"), lnba=rowf("bta"),
        lng1=rowf("g1"), lnb1=rowf("bt1"),
        lng2=rowf("g2"), lnb2=rowf("bt2"),
    )
    in_maps = []
    for c in range(NC):
        si = np.zeros((NB, 128, TB), np.int32)
        mt = np.zeros((NB, 128, TB, 128), bf16)
        m01 = np.zeros((NB, 128, TB, 128), bf16)
        for b in range(NB):
            g = c * NB + b
            s, e = starts[g], starts[g + 1]
            cnt = e - s
            esrc = src[s:e]
            edl = (dst[s:e] - (g * 128)).astype(np.int32)
            pad = TB * 128 - cnt
            esrc = np.concatenate([esrc, np.zeros(pad, np.int32)])
            edl = np.concatenate([edl, np.full(pad, 255, np.int32)])
            esrc = esrc.reshape(TB, 128)
            edl = edl.reshape(TB, 128)
            si[b] = esrc.T
            onehot = (edl[:, :, None] == np.arange(128)[None, None, :])
            mt[b] = onehot.transpose(1, 0, 2).astype(bf16)
            m01[b] = onehot.transpose(2, 0, 1).astype(bf16)
        m = dict(shared)
        m["xTl"] = np.ascontiguousarray(xT_b[:, c * ND:(c + 1) * ND])
        m["sidx"] = si
        m["mt_in"] = mt
        m["m01_in"] = m01
        in_maps.append(m)
    return in_maps


class _Runner:
    """Compile once; keep a reusable jitted sharded executable.

    Mirrors concourse.bass2jax.run_bass_via_pjrt's multi-core path, but
    memoizes the jit so repeated calls don't re-trace, and exposes a
    device-resident timing mode.
    """

    def __init__(self):
        import jax
        import concourse.mybir as mb
        from concourse import bass2jax
        from jax.sharding import Mesh, PartitionSpec, NamedSharding
        from jax.experimental.shard_map import shard_map

        bass2jax.install_neuronx_cc_hook()
        nc = build_program()
        self.nc = nc
        part_name = nc.partition_id_tensor.name if nc.partition_id_tensor else None
        in_names, out_names, out_avals, zero_shapes = [], [], [], []
        for alloc in nc.m.functions[0].allocations:
            if not isinstance(alloc, mb.MemoryLocationSet):
                continue
            name = alloc.memorylocations[0].name
            if alloc.kind == "ExternalInput":
                if name != part_name:
                    in_names.append(name)
            elif alloc.kind == "ExternalOutput":
                out_names.append(name)
                out_avals.append(jax.core.ShapedArray(
                    tuple(alloc.tensor_shape), mb.dt.np(alloc.dtype)))
                zero_shapes.append((tuple(alloc.tensor_shape), mb.dt.np(alloc.dtype)))
        n_params = len(in_names)
        n_outs = len(out_names)
        all_names = in_names + out_names
        if part_name is not None:
            all_names = all_names + [part_name]
        self.in_names = in_names
        self.out_names = out_names

        def _body(*args):
            operands = list(args)
            if part_name is not None:
                operands.append(bass2jax.partition_id_tensor())
            outs = bass2jax._bass_exec_p.bind(
                *operands,
                out_avals=tuple(out_avals),
                in_names=tuple(all_names),
                out_names=tuple(out_names),
                lowering_input_output_aliases=(),
                sim_require_finite=True,
                sim_require_nnan=True,
                nc=nc,
            )
            return tuple(outs)

        devices = jax.devices()[:NC]
        mesh = Mesh(np.asarray(devices), ("core",))
        self.mesh = mesh
        self.sharding = NamedSharding(mesh, PartitionSpec("core"))
        in_specs = (PartitionSpec("core"),) * (n_params + n_outs)
        out_specs = (PartitionSpec("core"),) * n_outs
        # outputs are fully written by the kernel, so no donation / zero-init
        # is needed; the zero args exist only to satisfy the parameter list.
        self.sharded = jax.jit(
            shard_map(_body, mesh=mesh, in_specs=in_specs, out_specs=out_specs,
                      check_rep=False),
            keep_unused=True)

        def _bodyK(K):
            def f(*args):
                outs = None
                for _ in range(K):
                    outs = _body(*args)
                return outs
            return f

        self._mk_chain = lambda K: jax.jit(
            shard_map(_bodyK(K), mesh=mesh, in_specs=in_specs,
                      out_specs=out_specs, check_rep=False),
            keep_unused=True)
        self._chains = {}

        import jax.numpy as jnp

        def _zeros():
            return tuple(jnp.zeros((NC * s[0], *s[1:]), d) for s, d in zero_shapes)

        self.zmaker = jax.jit(_zeros, out_shardings=(self.sharding,) * n_outs)
        self._zs = None
        self.jax = jax

    def put_inputs(self, in_maps):
        concat = [np.concatenate([np.asarray(in_maps[c][k]) for c in range(NC)], axis=0)
                  for k in self.in_names]
        return [self.jax.device_put(a, self.sharding) for a in concat]

    def zeros(self):
        if self._zs is None:
            self._zs = self.zmaker()
            self.jax.block_until_ready(self._zs)
        return self._zs

    def exec_dev(self, dev_inputs):
        outs = self.sharded(*dev_inputs, *self.zeros())
        self.jax.block_until_ready(outs)
        return outs

    def exec_chain(self, dev_inputs, K):
        if K not in self._chains:
            self._chains[K] = self._mk_chain(K)
        outs = self._chains[K](*dev_inputs, *self.zeros())
        self.jax.block_until_ready(outs)
        return outs

    def run(self, in_maps):
        dev_inputs = self.put_inputs(in_maps)
        outs = self.exec_dev(dev_inputs)
        res = []
        for c in range(NC):
            d = {}
            for i, name in enumerate(self.out_names):
                arr = np.asarray(outs[i])
                d[name] = arr.reshape(NC, -1, *arr.shape[1:])[c].reshape(
                    arr.shape[0] // NC, *arr.shape[1:])
            res.append(d)
        return res


def _get_runner():
    if "runner" not in _cache:
        _cache["runner"] = _Runner()
    return _cache["runner"]


def kernel(**inputs):
    runner = _get_runner()
    in_maps = host_prep(inputs)
    results = runner.run(in_maps)
    out = np.concatenate([results[c]["out"] for c in range(NC)], axis=0)
    return out
